# revision 12
# baseline (speedup 1.0000x reference)
"""Trainium2 Bass kernel for nn_BaselinePointerNetwork (CVRP pointer network,
encoder + 256-step autoregressive sampling decode), data-parallel over 8 cores.

Contract: kernel(**inputs) takes the FULL unsharded inputs (B=1024) and
returns (actions [B, T] int32, lps [B] float32), matching reference.py.

Strategy
--------
- Batch is sharded 128 instances per core; no cross-core communication.
- Gumbel noise for jax.random.categorical is precomputed host-side with
  jax (CPU) from the fixed key 42 — it depends only on constants, never on
  inputs — making device sampling argmax(logits + g_t) bit-exact vs the
  reference.
- Algebra: pin @ Wp1 splits into a step-invariant part U = att_emb @ Wp1[:H]
  + bp1 (precomputed once) and a per-step rank-1 part c2 = ctx @ Wp1[H:],
  maintained incrementally: AWsum = sum_unvisited(att_emb @ Wp1[H:]) is
  updated by subtracting the newly-visited node's row (gathered from DRAM
  by indirect DMA). This cuts per-step FLOPs ~65x vs the reference.
- Per-step layouts: heavy tensors [H=128 partitions, (n, i) free]; per-step
  state and sampling ops [inst=128 partitions, N free]. The hidden-layer
  contraction with Wp2 runs as 128 self-loading fp32 PE matmuls
  (h-chunk as stationary operand), which lands sc directly in [inst, n].
"""
import os
import sys

sys.path.insert(0, "/opt/trn_rl_repo")

import numpy as np

import bass_rust
import concourse.bass as bass
import concourse.tile as tile
from concourse import mybir
from concourse.masks import make_identity

f32 = mybir.dt.float32
i32 = mybir.dt.int32
u32 = mybir.dt.uint32
Alu = mybir.AluOpType
Act = mybir.ActivationFunctionType
Axis = mybir.AxisListType

B, N, H = 1024, 128, 128
T_FULL = 2 * N
NCORES = 8
BL = B // NCORES  # 128 instances per core
NEG = -1e9
INV_SQRT_H = float(1.0 / np.sqrt(np.float32(H), dtype=np.float32))


def split_excess_waits(nc, max_waits=1):
    """This walrus build rejects >1 semaphore wait per CTRL instruction;
    split excess waits onto preceding Drain instructions (same engine,
    executed in order, so semantics are preserved)."""
    n = 0
    for f in nc.m.functions:
        for blk in f.blocks:
            newl = []
            changed = False
            for inst in blk.instructions:
                si = inst.sync_info
                if si is not None and len(si.on_wait) > max_waits:
                    waits = list(si.on_wait)
                    rest = waits[-max_waits:]
                    head = waits[:-max_waits]
                    k = 0
                    while head:
                        chunk, head = head[:max_waits], head[max_waits:]
                        nd = mybir.InstDrain(
                            name=f"{inst.name}-wsplit{k}", engine=inst.engine,
                            ins=[], outs=[])
                        nd.sync_info = bass_rust.SyncInfo(
                            on_wait=chunk, on_update=[])
                        newl.append(nd)
                        k += 1
                        n += 1
                    inst.sync_info = bass_rust.SyncInfo(
                        on_wait=rest, on_update=list(si.on_update))
                    changed = True
                newl.append(inst)
            if changed:
                blk.instructions = newl
    return n


def gumbel_table():
    """[T, B, N] f32 gumbel noise, bit-exact vs jax.random.categorical's
    internal noise for key 42 (computed on the jax CPU backend)."""
    import jax

    cpu = jax.devices("cpu")[0]
    with jax.default_device(cpu):
        keys = jax.random.split(jax.random.key(42), T_FULL)
        out = np.empty((T_FULL, B, N), np.float32)
        for t in range(T_FULL):
            out[t] = np.asarray(jax.random.gumbel(keys[t], (B, N), np.float32))
    return out


def build_nc(bp2_val: float, t_steps: int):
    skip = set(os.environ.get("KM_SKIP", "").split(","))
    nc = bass.Bass()

    # ---------------- DRAM tensors ----------------
    nfT_d = nc.dram_tensor("nfT", [3, BL * N], f32, kind="ExternalInput")
    demands_d = nc.dram_tensor("demands", [BL, N], f32, kind="ExternalInput")
    caps_d = nc.dram_tensor("caps", [BL, 1], f32, kind="ExternalInput")
    W_emb_d = nc.dram_tensor("W_emb", [3, H], f32, kind="ExternalInput")
    Wq_d = nc.dram_tensor("Wq", [H, H], f32, kind="ExternalInput")
    Wk_d = nc.dram_tensor("Wk", [H, H], f32, kind="ExternalInput")
    Wv_d = nc.dram_tensor("Wv", [H, H], f32, kind="ExternalInput")
    W1a_d = nc.dram_tensor("W1a", [H, H], f32, kind="ExternalInput")
    W1b_d = nc.dram_tensor("W1b", [H, H], f32, kind="ExternalInput")
    Wp2_d = nc.dram_tensor("Wp2", [H, 1], f32, kind="ExternalInput")
    b_emb_d = nc.dram_tensor("b_emb", [H, 1], f32, kind="ExternalInput")
    bq_d = nc.dram_tensor("bq", [H, 1], f32, kind="ExternalInput")
    bk_d = nc.dram_tensor("bk", [H, 1], f32, kind="ExternalInput")
    bv_rep_d = nc.dram_tensor("bv_rep", [N, H], f32, kind="ExternalInput")
    bp1_d = nc.dram_tensor("bp1", [H, 1], f32, kind="ExternalInput")
    g_d = nc.dram_tensor("g", [T_FULL * BL, N], f32, kind="ExternalInput")

    aw_dram = nc.dram_tensor("aw_scratch", [BL * N, H], f32)

    actions_d = nc.dram_tensor("actions", [BL, T_FULL], i32, kind="ExternalOutput")
    lps_d = nc.dram_tensor("lps", [BL, 1], f32, kind="ExternalOutput")

    with tile.TileContext(nc) as tc:
        with (
            tc.tile_pool(name="cpool", bufs=1) as cp,
            tc.tile_pool(name="spool", bufs=3) as sp,
            tc.tile_pool(name="pspool", bufs=4, space="PSUM") as pp,
            tc.tile_pool(name="scps", bufs=2, space="PSUM") as pp_sc,
        ):
            # ---------------- persistent SBUF ----------------
            U = cp.tile([H, N * BL], f32)       # [h2, n*128+i]
            hbuf = cp.tile([H, N * BL], f32)    # relu'd hidden, same layout
            ident = cp.tile([128, 128], f32)
            indn_f = cp.tile([BL, N], f32)      # [i, n] = n
            iotap = cp.tile([BL, 1], i32)       # [i, 0] = i*128
            negcol = cp.tile([BL, 1], f32)      # -1e9
            demands = cp.tile([BL, N], f32)
            caps = cp.tile([BL, 1], f32)
            W_emb = cp.tile([3, H], f32)
            Wq = cp.tile([H, H], f32)
            Wk = cp.tile([H, H], f32)
            Wv = cp.tile([H, H], f32)
            W1a = cp.tile([H, H], f32)
            W1b = cp.tile([H, H], f32)
            Wp2 = cp.tile([H, 1], f32)
            b_emb = cp.tile([H, 1], f32)
            bq = cp.tile([H, 1], f32)
            bk = cp.tile([H, 1], f32)
            bv_rep = cp.tile([N, H], f32)
            bp1 = cp.tile([H, 1], f32)
            aesum = cp.tile([H, BL], f32)
            # decode state
            visited = cp.tile([BL, N], f32)
            awsum = cp.tile([BL, H], f32)
            rem_cap = cp.tile([BL, 1], f32)
            at_depot = cp.tile([BL, 1], f32)
            done = cp.tile([BL, 1], f32)
            cnt = cp.tile([BL, 1], f32)
            lps = cp.tile([BL, 1], f32)
            actions_sb = cp.tile([BL, T_FULL], i32)
            # decode scratch (persistent; serialized by deps within a step)
            gt = cp.tile([BL, N], f32)
            c2 = cp.tile([BL, H], f32)
            c2T = cp.tile([H, BL], f32)
            sc_sb = cp.tile([BL, N], f32)
            mask = cp.tile([BL, N], f32)
            logits = cp.tile([BL, N], f32)
            pert = cp.tile([BL, N], f32)
            onehot = cp.tile([BL, N], f32)
            scr = cp.tile([BL, N], f32)
            es = cp.tile([BL, N], f32)
            awg = cp.tile([BL, H], f32)
            awd = cp.tile([BL, H], f32)
            mx8 = cp.tile([BL, 8], f32)
            ix8 = cp.tile([BL, 8], u32)
            cols = {}
            for nm in ["nvis", "tav", "tav2", "rcm", "mask0p", "rmin", "am",
                       "t1", "t2", "uu", "nu", "mask0", "af", "selv", "negmx",
                       "sumexp", "lnse", "sel", "nd", "isd", "nisd", "newvis",
                       "dtk", "rem1", "rem2", "a_i32", "rowidx"]:
                cols[nm] = cp.tile([BL, 1],
                                   i32 if nm in ("a_i32", "rowidx") else f32,
                                   name=f"col_{nm}", tag=f"col_{nm}")

            # ---------------- init ----------------
            make_identity(nc, ident[:, :])
            indn_i = sp.tile([BL, N], i32, tag="indn_i")
            nc.gpsimd.iota(indn_i[:, :], pattern=[[1, N]], base=0, channel_multiplier=0)
            nc.vector.tensor_copy(indn_f[:, :], indn_i[:, :])
            nc.gpsimd.iota(iotap[:, :], pattern=[[0, 1]], base=0, channel_multiplier=N)
            nc.gpsimd.memset(negcol[:, :], NEG)
            nc.gpsimd.memset(visited[:, :], 0.0)
            nc.gpsimd.memset(at_depot[:, :], 1.0)
            nc.gpsimd.memset(done[:, :], 0.0)
            nc.gpsimd.memset(cnt[:, :], float(N))
            nc.gpsimd.memset(lps[:, :], 0.0)

            for t_, d_ in [(demands, demands_d), (caps, caps_d), (W_emb, W_emb_d),
                           (Wq, Wq_d), (Wk, Wk_d), (Wv, Wv_d), (W1a, W1a_d),
                           (W1b, W1b_d), (Wp2, Wp2_d), (b_emb, b_emb_d),
                           (bq, bq_d), (bk, bk_d), (bv_rep, bv_rep_d),
                           (bp1, bp1_d)]:
                nc.sync.dma_start(t_[:, :], d_[:, :])
            nc.sync.dma_start(rem_cap[:, :], caps_d[:, :])

            # ---------------- encoder (per instance) ----------------
            for i in range(BL):
                tok = slice(i * N, (i + 1) * N)
                nfT_i = sp.tile([3, N], f32, tag="nfT_i")
                nc.sync.dma_start(nfT_i[:, :], nfT_d[:, tok])
                e_ps = pp.tile([H, N], f32, tag="ps")
                nc.tensor.matmul(e_ps[:, :], lhsT=W_emb[:, :], rhs=nfT_i[:, :],
                                 start=True, stop=True)
                embT_i = sp.tile([H, N], f32, tag="embT")
                nc.scalar.activation(embT_i[:, :], e_ps[:, :], Act.Identity,
                                     bias=b_emb[:, 0:1])

                q_ps = pp.tile([H, N], f32, tag="ps")
                nc.tensor.matmul(q_ps[:, :], lhsT=Wq[:, :], rhs=embT_i[:, :],
                                 start=True, stop=True)
                qT_i = sp.tile([H, N], f32, tag="qT")
                nc.scalar.activation(qT_i[:, :], q_ps[:, :], Act.Identity,
                                     bias=bq[:, 0:1])

                k_ps = pp.tile([H, N], f32, tag="ps")
                nc.tensor.matmul(k_ps[:, :], lhsT=Wk[:, :], rhs=embT_i[:, :],
                                 start=True, stop=True)
                kT_i = sp.tile([H, N], f32, tag="kT")
                nc.scalar.activation(kT_i[:, :], k_ps[:, :], Act.Identity,
                                     bias=bk[:, 0:1])

                v_ps = pp.tile([N, H], f32, tag="ps")
                nc.tensor.matmul(v_ps[:, :], lhsT=embT_i[:, :], rhs=Wv[:, :],
                                 start=True, stop=True)
                V_i = sp.tile([N, H], f32, tag="V")
                nc.vector.tensor_tensor(V_i[:, :], v_ps[:, :], bv_rep[:, :], Alu.add)

                s_ps = pp.tile([N, N], f32, tag="ps")
                nc.tensor.matmul(s_ps[:, :], lhsT=qT_i[:, :], rhs=kT_i[:, :],
                                 start=True, stop=True)
                mxr = sp.tile([N, 1], f32, tag="mxr")
                nc.vector.tensor_reduce(mxr[:, :], s_ps[:, :], Axis.X, Alu.max)
                nbr = sp.tile([N, 1], f32, tag="nbr")
                nc.vector.tensor_scalar(nbr[:, :], mxr[:, :], -INV_SQRT_H, None,
                                        Alu.mult)
                attn = sp.tile([N, N], f32, tag="attn")
                rs = sp.tile([N, 1], f32, tag="rs")
                nc.scalar.activation(attn[:, :], s_ps[:, :], Act.Exp,
                                     bias=nbr[:, 0:1], scale=INV_SQRT_H,
                                     accum_out=rs[:, 0:1])
                rr = sp.tile([N, 1], f32, tag="rr")
                nc.vector.reciprocal(rr[:, :], rs[:, :])
                nc.vector.tensor_scalar(attn[:, :], attn[:, :], rr[:, 0:1], None,
                                        Alu.mult)

                t_ps = pp.tile([N, N], f32, tag="ps")
                nc.tensor.transpose(t_ps[:, :], attn[:, :], ident[:, :])
                attnT = sp.tile([N, N], f32, tag="attnT")
                nc.vector.tensor_copy(attnT[:, :], t_ps[:, :])

                ae_ps = pp.tile([H, N], f32, tag="ps")
                nc.tensor.matmul(ae_ps[:, :], lhsT=V_i[:, :], rhs=attnT[:, :],
                                 start=True, stop=True)
                ae_i = sp.tile([H, N], f32, tag="ae")
                nc.scalar.activation(ae_i[:, :], ae_ps[:, :], Act.Copy)

                u_ps = pp.tile([H, N], f32, tag="ps")
                nc.tensor.matmul(u_ps[:, :], lhsT=W1a[:, :], rhs=ae_i[:, :],
                                 start=True, stop=True)
                # U slab columns n*128 + i (strided)
                u_cols = U[:, :].rearrange("h (n i) -> h n i", i=BL)[:, :, i]
                nc.scalar.activation(u_cols, u_ps[:, :], Act.Identity,
                                     bias=bp1[:, 0:1])

                awt_ps = pp.tile([N, H], f32, tag="ps")
                nc.tensor.matmul(awt_ps[:, :], lhsT=ae_i[:, :], rhs=W1b[:, :],
                                 start=True, stop=True)
                aw_st = sp.tile([N, H], f32, tag="aw_st")
                nc.vector.tensor_copy(aw_st[:, :], awt_ps[:, :])
                nc.sync.dma_start(aw_dram[i * N:(i + 1) * N, :], aw_st[:, :])

                nc.vector.tensor_reduce(aesum[:, i:i + 1], ae_i[:, :], Axis.X,
                                        Alu.add)

            # AWsum0 = (sum_n ae)^T @ W1b, then transpose to [i, h2]
            aws_ps = pp.tile([H, BL], f32, tag="ps")
            nc.tensor.matmul(aws_ps[:, :], lhsT=W1b[:, :], rhs=aesum[:, :],
                             start=True, stop=True)
            awsT = sp.tile([H, BL], f32, tag="awsT")
            nc.vector.tensor_copy(awsT[:, :], aws_ps[:, :])
            awsT_ps = pp.tile([BL, H], f32, tag="ps")
            nc.tensor.transpose(awsT_ps[:, :], awsT[:, :], ident[:, :])
            nc.vector.tensor_copy(awsum[:, :], awsT_ps[:, :])

            # ---------------- decode loop ----------------
            g_rows = g_d  # [T*BL, N]

            for iv in range(t_steps):
                # gumbel slab for this step
                nc.sync.dma_start(gt[:, :], g_rows[iv * BL:(iv + 1) * BL, :])

                # done |= all(visited[:,1:]) & at_depot
                nc.vector.tensor_reduce(cols["nvis"][:, :], visited[:, 1:N],
                                        Axis.X, Alu.add)
                nc.vector.tensor_scalar(cols["tav"][:, :], cols["nvis"][:, :],
                                        float(N - 1), None, Alu.is_ge)
                nc.vector.tensor_tensor(cols["tav2"][:, :], cols["tav"][:, :],
                                        at_depot[:, :], Alu.mult)
                nc.vector.tensor_tensor(done[:, :], done[:, :],
                                        cols["tav2"][:, :], Alu.max)

                # c2 = AWsum / max(cnt,1) ; transpose to [h2, i]
                nc.vector.tensor_scalar(cols["rcm"][:, :], cnt[:, :], 1.0, None,
                                        Alu.max)
                nc.vector.reciprocal(cols["rcm"][:, :], cols["rcm"][:, :])
                nc.vector.tensor_scalar(c2[:, :], awsum[:, :], cols["rcm"][:, 0:1],
                                        None, Alu.mult)
                c2T_ps = pp.tile([H, BL], f32, tag="ps")
                nc.tensor.transpose(c2T_ps[:, :], c2[:, :], ident[:, :])
                nc.vector.tensor_copy(c2T[:, :], c2T_ps[:, :])

                # hidden = relu(U + c2T broadcast); sc = Wp2 . hidden
                NCHUNK = 4
                csz = (N * BL) // NCHUNK
                ngrp = N // NCHUNK
                u3 = U[:, :].rearrange("h (n i) -> h n i", i=BL)
                h3 = hbuf[:, :].rearrange("h (n i) -> h n i", i=BL)
                c2b = c2T[:, :].unsqueeze(1).broadcast_to([H, ngrp, BL])
                for c in range(NCHUNK):
                    nsl = slice(c * ngrp, (c + 1) * ngrp)
                    if "hadd" not in skip or c == 0:
                        nc.vector.tensor_tensor(h3[:, nsl, :], u3[:, nsl, :], c2b,
                                                Alu.add)
                    fsl = slice(c * csz, (c + 1) * csz)
                    if "hrelu" not in skip or c == 0:
                        nc.scalar.activation(hbuf[:, fsl], hbuf[:, fsl], Act.Relu)
                sc_ps = pp_sc.tile([BL, N], f32, tag="sc")
                for n in range(N if "scmm" not in skip else 1):
                    nc.tensor.matmul(sc_ps[:, n:n + 1],
                                     lhsT=hbuf[:, n * BL:(n + 1) * BL],
                                     rhs=Wp2[:, :], start=True, stop=True)
                nc.scalar.activation(sc_sb[:, :], sc_ps[:, :], Act.Identity,
                                     bias=bp2_val)

                # mask
                nc.vector.tensor_scalar(mask[:, :], demands[:, :],
                                        rem_cap[:, 0:1], None, Alu.is_gt)
                nc.vector.tensor_tensor(mask[:, :], mask[:, :], visited[:, :],
                                        Alu.max)
                nc.vector.tensor_tensor(cols["mask0p"][:, :], mask[:, 0:1],
                                        at_depot[:, :], Alu.max)
                nc.vector.tensor_reduce(cols["rmin"][:, :], mask[:, 1:N], Axis.X,
                                        Alu.min)
                nc.vector.tensor_tensor(cols["am"][:, :], cols["rmin"][:, :],
                                        cols["mask0p"][:, :], Alu.min)
                nc.vector.tensor_tensor(cols["t1"][:, :], cols["am"][:, :],
                                        at_depot[:, :], Alu.mult)
                nc.vector.tensor_tensor(done[:, :], done[:, :], cols["t1"][:, :],
                                        Alu.max)
                nc.vector.tensor_tensor(cols["t2"][:, :], cols["am"][:, :],
                                        cols["t1"][:, :], Alu.subtract)
                nc.vector.tensor_tensor(cols["uu"][:, :], cols["t2"][:, :],
                                        done[:, :], Alu.max)
                nc.vector.tensor_scalar(cols["nu"][:, :], cols["uu"][:, :], -1.0,
                                        1.0, Alu.mult, Alu.add)
                nc.vector.tensor_tensor(cols["mask0"][:, :], cols["mask0p"][:, :],
                                        cols["nu"][:, :], Alu.mult)

                # logits
                nc.vector.tensor_copy(logits[:, :], sc_sb[:, :])
                nc.vector.copy_predicated(logits[:, :], mask[:, :].bitcast(i32),
                                          negcol[:, 0:1].to_broadcast([BL, N]))
                nc.vector.tensor_copy(logits[:, 0:1], sc_sb[:, 0:1])
                nc.vector.copy_predicated(logits[:, 0:1], cols["mask0"][:, :].bitcast(i32),
                                          negcol[:, 0:1])

                # sample
                nc.vector.tensor_tensor(pert[:, :], logits[:, :], gt[:, :],
                                        Alu.add)
                nc.vector.max(mx8[:, :], pert[:, :])
                nc.vector.max_index(ix8[:, :], mx8[:, :], pert[:, :])
                nc.vector.tensor_copy(cols["af"][:, :], ix8[:, 0:1])

                # selected log-prob
                nc.vector.tensor_scalar(onehot[:, :], indn_f[:, :],
                                        cols["af"][:, 0:1], None, Alu.is_equal)
                nc.vector.scalar_tensor_tensor(scr[:, :], onehot[:, :], 0.0,
                                               logits[:, :], Alu.add, Alu.mult,
                                               accum_out=cols["selv"][:, 0:1])
                nc.vector.tensor_reduce(cols["negmx"][:, :], logits[:, :], Axis.X,
                                        Alu.max, negate=True)
                nc.scalar.activation(es[:, :], logits[:, :], Act.Exp,
                                     bias=cols["negmx"][:, 0:1],
                                     accum_out=cols["sumexp"][:, 0:1])
                nc.scalar.activation(cols["lnse"][:, :], cols["sumexp"][:, :],
                                     Act.Ln)
                nc.vector.tensor_tensor(cols["sel"][:, :], cols["selv"][:, :],
                                        cols["negmx"][:, :], Alu.add)
                nc.vector.tensor_tensor(cols["sel"][:, :], cols["sel"][:, :],
                                        cols["lnse"][:, :], Alu.subtract)
                nc.vector.tensor_scalar(cols["nd"][:, :], done[:, :], -1.0, 1.0,
                                        Alu.mult, Alu.add)
                nc.vector.tensor_tensor(cols["sel"][:, :], cols["sel"][:, :],
                                        cols["nd"][:, :], Alu.mult)
                nc.vector.tensor_tensor(lps[:, :], lps[:, :], cols["sel"][:, :],
                                        Alu.add)

                # action (zeroed when done), store
                nc.vector.tensor_tensor(cols["af"][:, :], cols["af"][:, :],
                                        cols["nd"][:, :], Alu.mult)
                nc.vector.tensor_copy(cols["a_i32"][:, :], cols["af"][:, :])
                nc.vector.tensor_copy(actions_sb[:, iv:iv + 1],
                                      cols["a_i32"][:, :])

                # state updates
                nc.vector.tensor_scalar(cols["isd"][:, :], cols["af"][:, :], 0.0,
                                        None, Alu.is_equal)
                nc.vector.tensor_scalar(cols["nisd"][:, :], cols["isd"][:, :],
                                        -1.0, 1.0, Alu.mult, Alu.add)
                nc.vector.tensor_tensor(cols["newvis"][:, :], cols["nd"][:, :],
                                        cols["nisd"][:, :], Alu.mult)
                nc.vector.tensor_scalar(scr[:, :], onehot[:, :],
                                        cols["newvis"][:, 0:1], None, Alu.mult)
                nc.vector.tensor_tensor(visited[:, :], visited[:, :], scr[:, :],
                                        Alu.max)
                nc.vector.scalar_tensor_tensor(es[:, :], onehot[:, :], 0.0,
                                               demands[:, :], Alu.add, Alu.mult,
                                               accum_out=cols["dtk"][:, 0:1])
                nc.vector.tensor_tensor(cols["rem1"][:, :], rem_cap[:, :],
                                        cols["dtk"][:, :], Alu.subtract)
                nc.vector.tensor_copy(cols["rem2"][:, :], cols["rem1"][:, :])
                nc.vector.copy_predicated(cols["rem2"][:, :], cols["isd"][:, :].bitcast(i32),
                                          caps[:, :])
                nc.vector.copy_predicated(rem_cap[:, :], cols["nd"][:, :].bitcast(i32),
                                          cols["rem2"][:, :])
                nc.vector.copy_predicated(at_depot[:, :], cols["nd"][:, :].bitcast(i32),
                                          cols["isd"][:, :])

                # AWsum -= AW[i*N + a_i] * newvis
                nc.vector.tensor_tensor(cols["rowidx"][:, :], cols["a_i32"][:, :],
                                        iotap[:, :], Alu.add)
                if "gather" not in skip:
                    nc.gpsimd.indirect_dma_start(
                        out=awg[:, :], out_offset=None, in_=aw_dram[:, :],
                        in_offset=bass.IndirectOffsetOnAxis(
                            ap=cols["rowidx"][:, 0:1], axis=0))
                nc.vector.tensor_scalar(awd[:, :], awg[:, :],
                                        cols["newvis"][:, 0:1], None, Alu.mult)
                nc.vector.tensor_tensor(awsum[:, :], awsum[:, :], awd[:, :],
                                        Alu.subtract)
                nc.vector.tensor_tensor(cnt[:, :], cnt[:, :],
                                        cols["newvis"][:, :], Alu.subtract)

            if t_steps < T_FULL:
                nc.gpsimd.memset(actions_sb[:, t_steps:T_FULL], 0)
            nc.sync.dma_start(actions_d[:, :], actions_sb[:, :])
            nc.sync.dma_start(lps_d[:, :], lps[:, :])

    return nc


def make_in_maps(inputs, g):
    """Shard inputs across cores; returns list of per-core input dicts."""
    coords = np.ascontiguousarray(inputs["coords"], np.float32)
    demands = np.ascontiguousarray(inputs["demands"], np.float32)
    caps = np.ascontiguousarray(inputs["capacities"], np.float32)
    Wp1 = np.ascontiguousarray(inputs["Wp1"], np.float32)
    bv_rep = np.broadcast_to(np.ascontiguousarray(inputs["bv"], np.float32), (N, H)).copy()
    in_maps = []
    for c in range(NCORES):
        bsl = slice(c * BL, (c + 1) * BL)
        nfT = np.stack([
            coords[bsl, :, 0].reshape(-1),
            coords[bsl, :, 1].reshape(-1),
            demands[bsl].reshape(-1),
        ]).astype(np.float32)
        in_maps.append({
            "nfT": np.ascontiguousarray(nfT),
            "demands": np.ascontiguousarray(demands[bsl]),
            "caps": np.ascontiguousarray(caps[bsl].reshape(BL, 1)),
            "W_emb": np.ascontiguousarray(inputs["W_emb"], np.float32),
            "Wq": np.ascontiguousarray(inputs["Wq"], np.float32),
            "Wk": np.ascontiguousarray(inputs["Wk"], np.float32),
            "Wv": np.ascontiguousarray(inputs["Wv"], np.float32),
            "W1a": np.ascontiguousarray(Wp1[:H], np.float32),
            "W1b": np.ascontiguousarray(Wp1[H:], np.float32),
            "Wp2": np.ascontiguousarray(inputs["Wp2"], np.float32),
            "b_emb": np.ascontiguousarray(inputs["b_emb"], np.float32).reshape(H, 1),
            "bq": np.ascontiguousarray(inputs["bq"], np.float32).reshape(H, 1),
            "bk": np.ascontiguousarray(inputs["bk"], np.float32).reshape(H, 1),
            "bv_rep": bv_rep,
            "bp1": np.ascontiguousarray(inputs["bp1"], np.float32).reshape(H, 1),
            "g": np.ascontiguousarray(
                g[:, bsl, :].reshape(T_FULL * BL, N)),
        })
    return in_maps


def kernel(**inputs):
    from concourse.bass_utils import run_bass_kernel_spmd

    t_steps = int(os.environ.get("KERNEL_T_STEPS", T_FULL))
    g = gumbel_table()
    nc = build_nc(bp2_val=float(np.float32(inputs["bp2"][0])), t_steps=t_steps)
    split_excess_waits(nc)
    in_maps = make_in_maps(inputs, g)
    trace = os.environ.get("KERNEL_TRACE", "0") == "1"
    res = run_bass_kernel_spmd(nc, in_maps, core_ids=list(range(NCORES)),
                               trace=trace)
    if trace:
        print("HW exec time:", res.exec_time_ns, "ns")
        print("trace:", res.instructions_and_trace[1] if res.instructions_and_trace else None)
    actions = np.concatenate([r["actions"] for r in res.results], 0)
    lps = np.concatenate([r["lps"][:, 0] for r in res.results], 0)
    return actions.astype(np.int32), lps.astype(np.float32)
